# revision 1
# baseline (speedup 1.0000x reference)
"""Trainium2 Bass kernel for nn_DiagonalTraining (anti-diagonal per-diag Linear).

out[b, r, c] = sum_{k} W[d, m, k] * x[b, r0(d)+k, d-r0(d)-k] + bias[d, m],
with d = r + c, m = r - r0(d).

Strategy: shard the 511 independent diagonals across 8 cores. All streams
are bf16 (rel-err budget 2e-2; measured ~2.4e-3), which halves HBM traffic
vs f32 and runs the PE at 1 cycle/row for any N.

Long diagonals (n > 128, d in [128,382], 255 of them) are grouped into
complementary PAIRS with nA + nB = 384 so the two chunk-1 k-ranges
(aA = nA-128, aB = nB-128, aA+aB = 128) exactly fill one 128-partition
stationary tile.  Per pair, 3 stationary xd tiles [128k x 128b] and 3
matmuls into one psum accumulation group over [0:NA+NB):
  merged chunk-1 pass first (opens the group over the full range, W1A|W1B
  are column-adjacent so one moving pass covers both), then the two
  chunk-0 passes accumulate into their sub-ranges.
W0A/W0B are sent at (near-)exact width; W1A/W1B carry structural zero rows
(the other pair member's partitions).

SPMD runs ONE program on 8 cores, so per-core column layouts must agree:
the 127 pairs + the standalone n=256 diagonal are sorted by size into 16
"slots"; slot u has one pair per core and a uniform (NA_u, NB_u) padded to
the slot max (pad <= 4 cols since sorted).

Short diagonals (n <= 128) keep the pair-packed block-diagonal bins of the
f32 baseline: 129 real bins + 7 dummies = 8 x 17, each one [128k x 128m]
matmul.

Outputs are written bf16, exact-packed, and unpacked/scattered on host.
Input DMAs split across the two HWDGE rings (SP + ACT) which stream
concurrently at ~430 GB/s aggregate; early output DMAs ride the GPSIMD
SWDGE ring, tail outputs the SP ring once its inputs have drained.
"""

import sys

sys.path.insert(0, "/opt/trn_rl_repo")

import numpy as np

B, S = 128, 256
D = 2 * S - 1  # 511
NCORES = 8
NSLOT = 16  # long pair-slots per core
NSB = 17  # short bins per core
NPS = 8  # psum banks cycled over jobs

TRACE = False  # test.py sets True to pull exec_time_ns from the NTFF profile
last_results = None

# Input DMAs ride the two HWDGE rings (sync=SP, scalar=ACT) so both queue
# rows stream concurrently (~430 GB/s aggregate vs ~282 on one), balanced
# at ~2.9 MB per ring; the final unit is a single small slot so the tail
# isn't latency-quantized by a big group.
# ("G", u0, u1) = slots u0..u1-1, ("D",) = the shorts stream, ("L", u).
RING_A = [("G", 0, 3), ("D", 0), ("D", 1), ("L", 12), ("L", 13), ("L", 14)]
RING_B = [("G", 3, 6), ("G", 6, 9), ("G", 9, 12), ("L", 15)]  # scalar
DSPLIT = 9  # shorts stream ships as bins [0,9) then [9,NSB)
N_JOBS = 16 + NSB  # 33


def _geom(d):
    r0 = max(0, d - S + 1)
    n = d + 1 if d < S else 2 * S - 1 - d
    return r0, n


def _layout():
    """Global slot structure: slots[u][c] = (dA, dB|None), uniform shapes."""
    pairs = [(d, 382 - d) for d in range(128, 191)]  # left: nA+nB = 384
    pairs += [(d, 638 - d) for d in range(320, 383)]  # right (A = smaller n)
    pairs.append((191, 319))  # the two n=192 diagonals
    pairs.sort(key=lambda p: -_geom(p[0])[1])  # by nA desc
    rslots = [pairs[7 + 8 * u : 15 + 8 * u] for u in range(15)]
    # standalone slot: 7 biggest pairs on cores 0-6, the n=256 diag on core 7
    sx = pairs[:7] + [(255, None)]
    # the wide standalone slot computes FIRST (PE is idle early); the job
    # order then descends by size so the tail holds only the smallest slots
    slots = [sx] + rslots

    shapes = []
    for ent in slots:
        NA = max(_geom(dA)[1] for dA, _ in ent)
        NB = max(_geom(dB)[1] if dB is not None else 0 for _, dB in ent)
        shapes.append((NA, NB))

    col0, CL = [], 0
    for NA, NB in shapes:
        col0.append(CL)
        CL += 384 + 2 * (NA + NB)
    ocol0, OL = [], 0
    for NA, NB in shapes:
        ocol0.append(OL)
        OL += NA + NB
    return slots, shapes, col0, CL, ocol0, OL


_SLOTS, _SHAPES, _COL0, CL, _OCOL0, OL = _layout()


def _short_bins():
    sbins = []
    for kk in range(1, 64):
        sbins.append([kk - 1, 127 - kk])
        sbins.append([511 - kk, 383 + kk])
    sbins.append([63, 447])
    sbins.append([127])
    sbins.append([383])
    sbins += [[] for _ in range(136 - len(sbins))]
    return sbins


def _wblk(d_, n_, koff, plo, phi, width):
    """W moving block [128, width]: [p, m] = W[d_, m, koff + p - plo]
    valid for p in [plo, phi) and m < n_; zero elsewhere."""
    p = np.arange(128)[:, None]
    m = np.arange(width)[None, :]
    kk = koff + (p - plo)
    msk = (p >= plo) & (p < phi) & (m < n_)
    idx = d_ * S * S + m * S + np.clip(kk, 0, S - 1)
    return np.where(msk, idx, 0).astype(np.int64), msk


def _diag_flat(d, kvals):
    """Flat x/grid index of diagonal d at positions kvals."""
    r0, n = _geom(d)
    r = r0 + kvals
    return r * S + (d - r)


def _core_tables():
    """Static per-core packing tables."""
    cores = []
    for c in range(NCORES):
        xdb = []  # (dstcol, idx[128], valid)
        wb = []  # (dstcol, idx[128, w], msk[128, w])
        tgt_l = np.full(OL, -1, np.int64)
        k = np.arange(128)
        for u in range(NSLOT):
            dA, dB = _SLOTS[u][c]
            NA, NB = _SHAPES[u]
            c0 = _COL0[u]
            r0A, nA = _geom(dA)
            aA = nA - 128
            xdb.append((c0, _diag_flat(dA, k), True))
            if dB is not None:
                r0B, nB = _geom(dB)
                xdb.append((c0 + 128, _diag_flat(dB, k), True))
            else:
                nB = 0
                xdb.append((c0 + 128, np.zeros(128, np.int64), False))
            # mixed chunk-1 stationary: p < aA -> A k=128+p, else B k=128+(p-aA)
            iA = _diag_flat(dA, np.minimum(128 + k, nA - 1))
            if dB is not None:
                iB = _diag_flat(dB, np.clip(128 + (k - aA), 0, nB - 1))
            else:
                iB = np.zeros(128, np.int64)
            xdb.append((c0 + 256, np.where(k < aA, iA, iB), True))
            # W moving blocks
            i0, m0 = _wblk(dA, nA, 0, 0, 128, NA)
            wb.append((c0 + 384, i0, m0))
            i1, m1 = _wblk(dB, nB, 0, 0, 128, NB) if dB is not None else (
                np.zeros((128, NB), np.int64), np.zeros((128, NB), bool))
            wb.append((c0 + 384 + NA, i1, m1))
            i2, m2 = _wblk(dA, nA, 128, 0, aA, NA)
            wb.append((c0 + 384 + NA + NB, i2, m2))
            i3, m3 = _wblk(dB, nB, 128, aA, 128, NB) if dB is not None else (
                np.zeros((128, NB), np.int64), np.zeros((128, NB), bool))
            wb.append((c0 + 384 + 2 * NA + NB, i3, m3))
            # output scatter targets
            tgt_l[_OCOL0[u] : _OCOL0[u] + nA] = _diag_flat(dA, np.arange(nA))
            if dB is not None:
                tgt_l[_OCOL0[u] + NA : _OCOL0[u] + NA + nB] = _diag_flat(
                    dB, np.arange(nB))

        # ---- short bins (same packing as the f32 baseline) ----
        sbins = _short_bins()
        my_s = sbins[c::NCORES]
        xds_i = np.zeros((NSB, 128), np.int64)
        xds_m = np.zeros((NSB, 128), np.float32)
        ws_i = np.zeros((NSB, 128, 128), np.int64)
        ws_m = np.zeros((NSB, 128, 128), np.float32)
        tgt_s = np.full((NSB, 128), -1, np.int64)
        for j, bin_ds in enumerate(my_s):
            off = 0
            for d in bin_ds:
                r0, n = _geom(d)
                i = np.arange(n)
                r = r0 + i
                col = d - r
                xds_i[j, off : off + n] = r * S + col
                xds_m[j, off : off + n] = 1.0
                ws_i[j, off : off + n, off : off + n] = (
                    d * S * S + i[None, :] * S + i[:, None]
                )
                ws_m[j, off : off + n, off : off + n] = 1.0
                tgt_s[j, off : off + n] = r * S + col
                off += n
        cores.append(
            dict(xdb=xdb, wb=wb, tgt_l=tgt_l, xds_i=xds_i, xds_m=xds_m,
                 ws_i=ws_i, ws_m=ws_m, tgt_s=tgt_s)
        )
    rr, cc = np.divmod(np.arange(S * S), S)
    dd = rr + cc
    r0v = np.maximum(0, dd - S + 1)
    bidx = dd * S + (rr - r0v)
    return cores, bidx


_TABLES = None
_PROG = None


def _tables():
    global _TABLES
    if _TABLES is None:
        _TABLES = _core_tables()
    return _TABLES


def _jobs():
    """Unified job order (matches cross-ring arrival order)."""
    jobs = [("L", u) for u in range(9)]
    jobs += [("S", j) for j in range(NSB)]
    jobs += [("L", u) for u in range(9, 16)]
    return jobs


def _cnt(k, e):
    """#copies on engine e (0=DVE, 1=ACT) among jobs 0..k (alternating)."""
    return (k + 2 - e) // 2 if k >= 0 else 0


def _build_program():
    import concourse.bass as bass
    import concourse.mybir as mybir

    f32 = mybir.dt.float32
    bf16 = mybir.dt.bfloat16
    nc = bass.Bass()
    dl = nc.dram_tensor("dl", [128, CL], bf16, kind="ExternalInput")
    ds = nc.dram_tensor("ds", [128, NSB * 256], bf16, kind="ExternalInput")
    yl = nc.dram_tensor("yl", [128, OL], bf16, kind="ExternalOutput")
    ys = nc.dram_tensor("ys", [128, NSB * 128], bf16, kind="ExternalOutput")

    # staging (one tensor per input DMA -> no WAR deps)
    def _slot_cols(u):
        return 384 + 2 * sum(_SHAPES[u])

    BTG = [
        nc.alloc_sbuf_tensor(
            f"btg{g}", [128, _COL0[u1 - 1] + _slot_cols(u1 - 1) - _COL0[u0]], bf16
        ).ap()
        for g, (u0, u1) in enumerate([(0, 3), (3, 6), (6, 9), (9, 12)])
    ]
    BTL = {
        u: nc.alloc_sbuf_tensor(f"btl{u}", [128, _slot_cols(u)], bf16).ap()
        for u in range(12, NSLOT)
    }
    BTS = nc.alloc_sbuf_tensor("bts", [128, NSB * 256], bf16).ap()
    YL = nc.alloc_sbuf_tensor("YL", [128, OL], bf16).ap()
    YS = nc.alloc_sbuf_tensor("YS", [128, NSB * 128], bf16).ap()
    PS = [nc.alloc_psum_tensor(f"ps{i}", [128, 512], f32).ap() for i in range(NPS)]

    # one DIN sem per input DMA; slot/shorts -> sem resolved via _job_sem
    DING = [nc.alloc_semaphore(f"dg{g}") for g in range(4)]
    DINL = {u: nc.alloc_semaphore(f"dl{u}") for u in range(12, NSLOT)}
    DINS = [nc.alloc_semaphore(f"dsm{i}") for i in range(2)]
    P = nc.alloc_semaphore("P")
    CV = nc.alloc_semaphore("CV")  # DVE copy completions (even jobs)
    CA = nc.alloc_semaphore("CA")  # ACT copy completions (odd jobs)
    DO = nc.alloc_semaphore("DO")

    jobs = _jobs()

    def _job_sem(kind, idx):
        if kind == "S":
            return DINS[0 if idx < DSPLIT else 1]
        if idx < 12:
            return DING[idx // 3]
        return DINL[idx]

    # (last-job-index, tensor, col range, ring) — early outs ride the SWDGE
    # ring (HWDGE rings are busy with inputs); tail outs ride the HWDGE rings
    out_events = [
        (5, "yl", 0, _OCOL0[6], "gpsimd"),
        (8, "yl", _OCOL0[6], _OCOL0[9], "gpsimd"),
        (8 + NSB, "ys", 0, NSB * 128, "sync"),
        (NSB + 11, "yl", _OCOL0[9], _OCOL0[12], "sync"),
        (NSB + 13, "yl", _OCOL0[12], _OCOL0[14], "sync"),
        (N_JOBS - 1, "yl", _OCOL0[14], OL, "sync"),
    ]

    def _in_dma(eng, item):
        if item[0] == "G":
            _, u0, u1 = item
            g = u0 // 3
            eng.dma_start(
                out=BTG[g][:],
                in_=dl[:, _COL0[u0] : _COL0[u1 - 1] + _slot_cols(u1 - 1)],
            ).then_inc(DING[g], 16)
        elif item[0] == "L":
            u = item[1]
            eng.dma_start(
                out=BTL[u][:], in_=dl[:, _COL0[u] : _COL0[u] + _slot_cols(u)]
            ).then_inc(DINL[u], 16)
        else:
            i = item[1]
            c0 = 0 if i == 0 else DSPLIT * 256
            c1 = DSPLIT * 256 if i == 0 else NSB * 256
            eng.dma_start(out=BTS[:, c0:c1], in_=ds[:, c0:c1]).then_inc(DINS[i], 16)

    def _out_dma(eng, ev):
        k, which, o0, o1, _ = ev
        eng.wait_ge(CV, _cnt(k, 0))
        eng.wait_ge(CA, _cnt(k, 1))
        t, st = (yl, YL) if which == "yl" else (ys, YS)
        eng.dma_start(out=t[:, o0:o1], in_=st[:, o0:o1]).then_inc(DO, 16)

    def _copy(eng, sem, ji, kind, idx):
        eng.wait_ge(P, ji + 1)
        ps = PS[ji % NPS]
        if kind == "L":
            NA, NB = _SHAPES[idx]
            o = _OCOL0[idx]
            if eng is nc.vector:
                cp = eng.tensor_copy(YL[:, o : o + NA + NB], ps[:, 0 : NA + NB])
            else:
                cp = eng.copy(YL[:, o : o + NA + NB], ps[:, 0 : NA + NB])
        else:
            dst = YS[:, idx * 128 : (idx + 1) * 128]
            if eng is nc.vector:
                cp = eng.tensor_copy(dst, ps[:, 0:128])
            else:
                cp = eng.copy(dst, ps[:, 0:128])
        cp.then_inc(sem, 1)

    with nc.Block(no_gpsimd_drain=True) as block:

        @block.sync
        def _(sync):
            for item in RING_A:
                _in_dma(sync, item)
            for ev in out_events:
                if ev[4] == "sync":
                    _out_dma(sync, ev)
            sync.wait_ge(DO, 16 * len(out_events))

        @block.gpsimd
        def _(gpsimd):
            # early output DMAs on the SWDGE ring (3rd concurrent queue row)
            for ev in out_events:
                if ev[4] == "gpsimd":
                    _out_dma(gpsimd, ev)

        @block.scalar
        def _(scalar):
            for item in RING_B:
                _in_dma(scalar, item)
            for ji, (kind, idx) in enumerate(jobs):
                if ji % 2 == 1:
                    _copy(nc.scalar, CA, ji, kind, idx)
                for ev in out_events:
                    if ev[4] == "scalar" and ev[0] == ji:
                        _out_dma(scalar, ev)

        @block.tensor
        def _(tensor):
            waited = set()
            for ji, (kind, idx) in enumerate(jobs):
                sem = _job_sem(kind, idx)
                if id(sem) not in waited:
                    tensor.wait_ge(sem, 16)
                    waited.add(id(sem))
                if ji >= NPS:
                    prev = ji - NPS
                    tensor.wait_ge(CV if prev % 2 == 0 else CA, _cnt(prev, prev % 2))
                ps = PS[ji % NPS]
                if kind == "L":
                    NA, NB = _SHAPES[idx]
                    if idx < 12:
                        bt = BTG[idx // 3]
                        o = _COL0[idx] - _COL0[(idx // 3) * 3]
                    else:
                        bt = BTL[idx]
                        o = 0
                    NT = NA + NB
                    xa = bt[:, o : o + 128]
                    xb = bt[:, o + 128 : o + 256]
                    xp = bt[:, o + 256 : o + 384]
                    wA0 = bt[:, o + 384 : o + 384 + NA]
                    wB0 = bt[:, o + 384 + NA : o + 384 + NT]
                    w1 = bt[:, o + 384 + NT : o + 384 + 2 * NT]
                    # W1A|W1B are column-adjacent: one moving pass covers both.
                    # It opens the accumulation group over the full [0:NT) so
                    # the chunk-0 passes accumulate into sub-ranges (a single
                    # group per bank — interleaved groups misaccumulate on HW)
                    nc.tensor.matmul(
                        ps[:, 0:NT], xp, w1, start=True, stop=False,
                        skip_group_check=True,
                    )
                    nc.tensor.matmul(
                        ps[:, 0:NA], xa, wA0, start=False, stop=False,
                        skip_group_check=True,
                    )
                    mm = nc.tensor.matmul(
                        ps[:, NA:NT], xb, wB0, start=False, stop=True,
                        skip_group_check=True,
                    )
                else:
                    o = idx * 256
                    mm = nc.tensor.matmul(
                        ps[:, 0:128],
                        BTS[:, o : o + 128],
                        BTS[:, o + 128 : o + 256],
                        start=True,
                        stop=True,
                    )
                mm.then_inc(P, 1)

        @block.vector
        def _(vector):
            for ji, (kind, idx) in enumerate(jobs):
                if ji % 2 == 0:
                    _copy(nc.vector, CV, ji, kind, idx)

    return nc


def _get_program():
    global _PROG
    if _PROG is None:
        _PROG = _build_program()
    return _PROG


def _pack_core(t, x_flat, W_flat, np_bf16):
    dl = np.zeros((128, CL), np.float32)
    for c0, idx, valid in t["xdb"]:
        if valid:
            dl[:, c0 : c0 + 128] = x_flat[:, idx].T
    for c0, idx, msk in t["wb"]:
        w = idx.shape[1]
        if w:
            dl[:, c0 : c0 + w] = W_flat[idx] * msk
    xds = x_flat[:, t["xds_i"]] * t["xds_m"]  # [B, NSB, 128]
    ws = W_flat[t["ws_i"]] * t["ws_m"]  # [NSB, 128k, 128m]
    dsb = np.zeros((128, NSB * 256), np.float32)
    dsb3 = dsb.reshape(128, NSB, 256)
    dsb3[:, :, 0:128] = xds.transpose(2, 1, 0)
    dsb3[:, :, 128:256] = ws.transpose(1, 0, 2)
    return {"dl": dl.astype(np_bf16), "ds": dsb.astype(np_bf16)}


def kernel(x, W, b):
    import ml_dtypes
    from concourse.bass_utils import run_bass_kernel_spmd

    x = np.asarray(x, np.float32)
    W = np.asarray(W, np.float32)
    b = np.asarray(b, np.float32)
    cores, bidx = _tables()
    x_flat = x.reshape(B, S * S)
    W_flat = W.reshape(-1)
    np_bf16 = ml_dtypes.bfloat16
    in_maps = [_pack_core(t, x_flat, W_flat, np_bf16) for t in cores]
    nc = _get_program()
    res = run_bass_kernel_spmd(nc, in_maps, core_ids=list(range(NCORES)), trace=TRACE)
    global last_results
    last_results = res
    out_flat = np.zeros((B, S * S), np.float32)
    for c, t in enumerate(cores):
        ylv = np.asarray(res.results[c]["yl"], np.float32).reshape(B, -1)
        fl = t["tgt_l"]
        vl = fl >= 0
        out_flat[:, fl[vl]] = ylv[:, vl]
        ysv = np.asarray(res.results[c]["ys"], np.float32).reshape(B, -1)
        fs = t["tgt_s"].reshape(-1)
        vs = fs >= 0
        out_flat[:, fs[vs]] = ysv[:, vs]
    out_flat += b.reshape(-1)[bidx][None, :]
    return out_flat.reshape(B, S, S)



# revision 10
# speedup vs baseline: 1.0374x; 1.0374x over previous
"""Trainium2 Bass kernel for nn_DiagonalTraining (anti-diagonal per-diag Linear).

out[b, r, c] = sum_k W[d, m, k] * xd[b, d, k] + bias[d, m],  d = r + c.

511 independent diagonals (lengths 1..256..1) sharded over 8 cores. All
streams bf16. The design minimizes HBM bytes (the kernel is DMA-bound at
~420 GB/s/core aggregate) and keeps both HWDGE rings streaming gap-free:

- Long diags n in [129,192]: SAME-LENGTH pairs (d, 510-d). Chunk-0 (k<128)
  x/W blocks are dense full-partition tiles; both chunk-1 residuals (a =
  n-128 rows each) stack into ONE dense partial-partition block [2a,
  128+N] (x residual + W residual side by side) shipped at full bandwidth
  with zero padding. The pair's two psum groups are sequential (A opens+
  closes, then B) so no interleaved-group hazard.
- Long diags n in [193,256]: standalone slots, chunk-1 as [a, 128+N]
  partial blocks, same trick.
- Shorts (n<=128): pair-packed block-diagonal bins as the baseline
  (x [128,128] stationary, W [128,128] moving, one matmul per bin).

Inputs stream first on the two HWDGE rings (greedy byte-balanced, in job
order); outputs are staged bf16 in SBUF and drain on the SWDGE ring
(gpsimd) as soon as their jobs' copies land, with the remainder on the
HWDGE rings once inputs finish. The last chunk is small so the tail is
short. Jobs execute in simulated-arrival order cycling 8 psum banks;
psum->SBUF copies alternate DVE/DVE/ACT.
"""

import sys

sys.path.insert(0, "/opt/trn_rl_repo")

import numpy as np

B, S = 128, 256
D = 2 * S - 1  # 511
NCORES = 8
NPS = 8  # psum banks cycled over jobs

TRACE = False  # test.py sets True to pull exec_time_ns from the NTFF profile
last_results = None


def _geom(d):
    r0 = max(0, d - S + 1)
    n = d + 1 if d < S else 2 * S - 1 - d
    return r0, n


def _diag_flat(d, kvals):
    r0, n = _geom(d)
    r = r0 + kvals
    return r * S + (d - r)


def _short_bins():
    sbins = []
    for kk in range(1, 64):
        sbins.append([kk - 1, 127 - kk])
        sbins.append([511 - kk, 383 + kk])
    sbins.append([63, 447])
    sbins.append([127])
    sbins.append([383])
    sbins += [[] for _ in range(136 - len(sbins))]
    return sbins


def _layout():
    """Static slot structure + schedule, shared by all cores (SPMD).

    Returns (units, jobs, CF, CP, OT, sbins).
    units: DMA units in per-ring issue order:
      {kind: 'F'|'P'|'SH', ring: 0|1, rows, cols, off (df/dp col offset)}
    jobs: execution-ordered:
      SN: {t:'SN', slot pairs[(dL,dR,n_c)] per core, N, a, fu, fo, pu, po,
           yo, w}
      ST: {t:'ST', diags[d|None], N, a, fu, fo, pu, po, yo, w}
      SH: {t:'SH', bin (global bin base idx), fu, fo, yo, w}
    """
    # ---- same-n pairs: n in [129,192] ----
    snp = [(n - 1, 511 - n, n) for n in range(192, 128, -1)]  # 64 pairs
    sn_slots = [snp[8 * u : 8 * u + 8] for u in range(8)]
    sn_N = [s[0][2] for s in sn_slots]
    # ---- standalone longs: n in [193,256] ----
    st = [255]  # n=256
    for n in range(255, 192, -1):
        st += [n - 1, 511 - n]
    st_slots = [st[8 * v : 8 * v + 8] for v in range(16)]
    st_slots[15] = st_slots[15] + [None] * (8 - len(st_slots[15]))
    st_N = [_geom(s[0])[1] for s in st_slots]
    # ---- shorts ----
    sbins = _short_bins()

    # ---- DMA units ----
    units = []

    def add_unit(kind, rows, cols):
        units.append(dict(kind=kind, rows=rows, cols=cols, ring=-1, off=0))
        return len(units) - 1

    # F units: SN groups of 4 slots, ST groups of 4, SH split in 2
    fu_sn = [add_unit("F", 128, sum(256 + 2 * sn_N[u] for u in range(4 * g, 4 * g + 4)))
             for g in range(2)]
    fu_st = [add_unit("F", 128, sum(128 + st_N[v] for v in range(4 * g, 4 * g + 4)))
             for g in range(4)]
    fu_sh = [add_unit("F", 128, 9 * 256), add_unit("F", 128, 8 * 256)]
    # P units: pairs of slots. Matmul operand base partitions must be in
    # {0, 32, 64}, so the B-half of an SN P-block sits at beta =
    # 32*ceil(a/32) instead of a (a few zero rows of padding).
    sn_beta = [32 * ((sn_N[u] - 128 + 31) // 32) for u in range(8)]
    pu_sn = [add_unit("P", sn_beta[2 * g] + (sn_N[2 * g] - 128),
                      (128 + sn_N[2 * g]) + (128 + sn_N[2 * g + 1]))
             for g in range(4)]
    pu_st = [add_unit("P", st_N[2 * g] - 128,
                      (128 + st_N[2 * g]) + (128 + st_N[2 * g + 1]))
             for g in range(8)]

    # df/dp col offsets
    cf = cp = 0
    for un in units:
        if un["kind"] == "P":
            un["off"] = cp
            cp += un["cols"]
        else:
            un["off"] = cf
            cf += un["cols"]
    CF, CP = cf, cp

    # ---- ring assignment: greedy balance in consumption order ----
    # consumption order of units: interleave F and matching P groups
    order = [
        fu_sn[0], pu_sn[0], pu_sn[1], fu_sn[1], pu_sn[2], pu_sn[3],
        fu_st[0], pu_st[0], pu_st[1], fu_st[1], pu_st[2], pu_st[3],
        fu_st[2], pu_st[4], pu_st[5], fu_st[3], pu_st[6], pu_st[7],
        fu_sh[0], fu_sh[1],
    ]
    rb = [0, 0]
    for ui in order:
        un = units[ui]
        r = 0 if rb[0] <= rb[1] else 1
        un["ring"] = r
        rb[r] += un["rows"] * un["cols"] * 2
    ring_units = [[ui for ui in order if units[ui]["ring"] == r] for r in (0, 1)]

    # ---- jobs ----
    jobs = []
    # per-slot F col offsets inside their unit
    sn_fo, off = [], [0, 0]
    for u in range(8):
        g = u // 4
        sn_fo.append(off[g])
        off[g] += 256 + 2 * sn_N[u]
    st_fo, off = [], [0, 0, 0, 0]
    for v in range(16):
        g = v // 4
        st_fo.append(off[g])
        off[g] += 128 + st_N[v]
    sn_po, off = [], [0, 0, 0, 0]
    for u in range(8):
        g = u // 2
        sn_po.append(off[g])
        off[g] += 128 + sn_N[u]
    st_po, off = [], [0] * 8
    for v in range(16):
        g = v // 2
        st_po.append(off[g])
        off[g] += 128 + st_N[v]

    for u in range(8):
        jobs.append(dict(t="SN", pairs=sn_slots[u], N=sn_N[u], a=sn_N[u] - 128,
                         beta=sn_beta[u], fu=fu_sn[u // 4], fo=sn_fo[u],
                         pu=pu_sn[u // 2], po=sn_po[u], w=2 * sn_N[u]))
    for v in range(16):
        jobs.append(dict(t="ST", diags=st_slots[v], N=st_N[v], a=st_N[v] - 128,
                         fu=fu_st[v // 4], fo=st_fo[v], pu=pu_st[v // 2],
                         po=st_po[v], w=st_N[v]))
    for j in range(17):
        jobs.append(dict(t="SH", bin=j, fu=fu_sh[0] if j < 9 else fu_sh[1],
                         fo=(j if j < 9 else j - 9) * 256, w=128))

    # ---- execution order = simulated arrival order ----
    # arrival of a unit = cumulative bytes before it on its ring (equal rates)
    arr = {}
    for r in (0, 1):
        c = 0
        for ui in ring_units[r]:
            un = units[ui]
            c += un["rows"] * un["cols"] * 2
            arr[ui] = c
    for k, jb in enumerate(jobs):
        a1 = arr[jb["fu"]]
        a2 = arr[jb["pu"]] if "pu" in jb else 0
        jb["arr"] = max(a1, a2)
        jb["tie"] = k
    jobs.sort(key=lambda jb: (jb["arr"], jb["tie"]))

    # yo offsets in execution order
    ot = 0
    for jb in jobs:
        jb["yo"] = ot
        ot += jb["w"]
    OT = ot
    return units, jobs, ring_units, CF, CP, OT, sbins


_UNITS, _JOBS, _RING_UNITS, CF, CP, OT, _SBINS = _layout()
N_JOBS = len(_JOBS)

# output chunks: (last_job_idx_inclusive, ring) ring: 'g'=SWDGE, 0, 1
_OUT_CHUNKS = [
    (7, "g"),
    (15, "g"),
    (23, "g"),
    (30, 1),
    (36, 0),
    (N_JOBS - 1, 1),
]


def _copy_eng(k):
    """0 = DVE, 1 = ACT. DVE takes 2 of 3 (ACT also issues ring-1 DMAs)."""
    return 0 if k % 3 != 2 else 1


def _cnt(k, e):
    """#copies on engine e among jobs 0..k inclusive."""
    return sum(1 for j in range(k + 1) if _copy_eng(j) == e)


def _core_tables():
    """Per-core packing index tables (host-side).

    xgath entries: (tensor 'df'|'dp', prow, col, idx[rows] into x_flat):
      image[prow:prow+rows, col:col+128... no -- writes
      image[prow:prow+len(idx), col:col+B] = x_flat[:, idx].T
    wblk entries: (tensor, prow, col, d, m0, m1, k0, k1):
      image[prow:prow+(k1-k0), col:col+(m1-m0)] = W[d, m0:m1, k0:k1].T
    """
    cores = []
    sbins = _SBINS
    for c in range(NCORES):
        my_bins = sbins[c::NCORES]
        xgath = []
        wblk = []
        tgt = np.full(OT, -1, np.int64)
        k128 = np.arange(128)
        for jb in _JOBS:
            if jb["t"] == "SN":
                dL, dR, n_c = jb["pairs"][c]
                N, a = jb["N"], jb["a"]
                fof = _UNITS[jb["fu"]]["off"] + jb["fo"]
                pof = _UNITS[jb["pu"]]["off"] + jb["po"]
                xgath.append(("df", 0, fof, _diag_flat(dL, k128)))
                xgath.append(("df", 0, fof + 128, _diag_flat(dR, k128)))
                wblk.append(("df", 0, fof + 256, dL, 0, N, 0, 128))
                wblk.append(("df", 0, fof + 256 + N, dR, 0, N, 0, 128))
                # P block: rows [0:a) A-chunk1, rows [beta:beta+a) B-chunk1
                bta = jb["beta"]
                kk = np.minimum(128 + k128[:a], n_c - 1)
                xgath.append(("dp", 0, pof, _diag_flat(dL, kk)))
                xgath.append(("dp", bta, pof, _diag_flat(dR, kk)))
                wblk.append(("dp", 0, pof + 128, dL, 0, N, 128, 128 + a))
                wblk.append(("dp", bta, pof + 128, dR, 0, N, 128, 128 + a))
                tgt[jb["yo"]: jb["yo"] + n_c] = _diag_flat(dL, np.arange(n_c))
                tgt[jb["yo"] + N: jb["yo"] + N + n_c] = _diag_flat(dR, np.arange(n_c))
            elif jb["t"] == "ST":
                d = jb["diags"][c]
                if d is None:
                    continue
                N, a = jb["N"], jb["a"]
                fof = _UNITS[jb["fu"]]["off"] + jb["fo"]
                pof = _UNITS[jb["pu"]]["off"] + jb["po"]
                _, n_c = _geom(d)
                a_c = n_c - 128
                xgath.append(("df", 0, fof, _diag_flat(d, k128)))
                wblk.append(("df", 0, fof + 128, d, 0, N, 0, 128))
                kk = np.minimum(128 + k128[:a_c], n_c - 1)
                xgath.append(("dp", 0, pof, _diag_flat(d, kk)))
                wblk.append(("dp", 0, pof + 128, d, 0, N, 128, 128 + a_c))
                tgt[jb["yo"]: jb["yo"] + n_c] = _diag_flat(d, np.arange(n_c))
            else:  # SH
                bin_ds = my_bins[jb["bin"]]
                base = _UNITS[jb["fu"]]["off"] + jb["fo"]
                o = 0
                for d in bin_ds:
                    _, n = _geom(d)
                    i = np.arange(n)
                    xgath.append(("df", o, base, _diag_flat(d, i)))
                    wblk.append(("df", o, base + 128 + o, d, 0, n, 0, n))
                    tgt[jb["yo"] + o: jb["yo"] + o + n] = _diag_flat(d, i)
                    o += n
        cores.append(dict(xgath=xgath, wblk=wblk, tgt=tgt))
    rr, cc = np.divmod(np.arange(S * S), S)
    dd = rr + cc
    r0v = np.maximum(0, dd - S + 1)
    bidx = dd * S + (rr - r0v)
    return cores, bidx


_TABLES = None
_PROG = None


def _tables():
    global _TABLES
    if _TABLES is None:
        _TABLES = _core_tables()
    return _TABLES


def _build_program():
    import concourse.bass as bass
    import concourse.mybir as mybir

    f32 = mybir.dt.float32
    bf16 = mybir.dt.bfloat16
    nc = bass.Bass()
    df = nc.dram_tensor("df", [128, CF], bf16, kind="ExternalInput")
    dp = nc.dram_tensor("dp", [128, CP], bf16, kind="ExternalInput")
    yo = nc.dram_tensor("yo", [128, OT], bf16, kind="ExternalOutput")

    # one SBUF staging tensor per DMA unit (no WAR deps)
    BT = [
        nc.alloc_sbuf_tensor(f"bt{i}", [128, un["cols"]], bf16).ap()
        for i, un in enumerate(_UNITS)
    ]
    YO = nc.alloc_sbuf_tensor("YO", [128, OT], bf16).ap()
    PS = [nc.alloc_psum_tensor(f"ps{i}", [128, 512], f32).ap() for i in range(NPS)]

    DIN = [nc.alloc_semaphore(f"di{i}") for i in range(len(_UNITS))]
    P = nc.alloc_semaphore("P")
    CV = nc.alloc_semaphore("CV")
    CA = nc.alloc_semaphore("CA")
    DO = nc.alloc_semaphore("DO")

    def _in_dma(eng, ui):
        un = _UNITS[ui]
        src = df if un["kind"] != "P" else dp
        r = un["rows"]
        eng.dma_start(
            out=BT[ui][0:r, :],
            in_=src[0:r, un["off"]: un["off"] + un["cols"]],
        ).then_inc(DIN[ui], 16)

    def _out_dma(eng, ev):
        k, _, o0, o1 = ev
        eng.wait_ge(CV, _cnt(k, 0))
        eng.wait_ge(CA, _cnt(k, 1))
        eng.dma_start(out=yo[:, o0:o1], in_=YO[:, o0:o1]).then_inc(DO, 16)

    # resolve chunk col ranges (jobs' yo offsets are in execution order)
    out_events = []
    prev = 0
    for k, ring in _OUT_CHUNKS:
        o1 = _JOBS[k]["yo"] + _JOBS[k]["w"]
        out_events.append((k, ring, prev, o1))
        prev = o1

    def _copy(eng, sem, k):
        eng.wait_ge(P, k + 1)
        jb = _JOBS[k]
        ps = PS[k % NPS]
        dst = YO[:, jb["yo"]: jb["yo"] + jb["w"]]
        if eng is nc.vector:
            cp = eng.tensor_copy(dst, ps[:, 0: jb["w"]])
        else:
            cp = eng.copy(dst, ps[:, 0: jb["w"]])
        cp.then_inc(sem, 1)

    with nc.Block(no_gpsimd_drain=True) as block:

        @block.sync
        def _(sync):
            for ui in _RING_UNITS[0]:
                _in_dma(sync, ui)
            for ev in out_events:
                if ev[1] == 0:
                    _out_dma(sync, ev)
            sync.wait_ge(DO, 16 * len(out_events))

        @block.gpsimd
        def _(gpsimd):
            for ev in out_events:
                if ev[1] == "g":
                    _out_dma(gpsimd, ev)

        @block.scalar
        def _(scalar):
            for ui in _RING_UNITS[1]:
                _in_dma(scalar, ui)
            for k in range(N_JOBS):
                if _copy_eng(k) == 1:
                    _copy(nc.scalar, CA, k)
                for ev in out_events:
                    if ev[1] == 1 and ev[0] == k:
                        _out_dma(scalar, ev)

        @block.vector
        def _(vector):
            for k in range(N_JOBS):
                if _copy_eng(k) == 0:
                    _copy(nc.vector, CV, k)

        @block.tensor
        def _(tensor):
            waited = set()
            for k, jb in enumerate(_JOBS):
                need = [jb["fu"]] + ([jb["pu"]] if "pu" in jb else [])
                for ui in need:
                    if ui not in waited:
                        tensor.wait_ge(DIN[ui], 16)
                        waited.add(ui)
                if k >= NPS:
                    prev_k = k - NPS
                    e = _copy_eng(prev_k)
                    tensor.wait_ge(CV if e == 0 else CA, _cnt(prev_k, e))
                ps = PS[k % NPS]
                if jb["t"] == "SN":
                    N, a, bta = jb["N"], jb["a"], jb["beta"]
                    F = BT[jb["fu"]]
                    Pp = BT[jb["pu"]]
                    fo, po = jb["fo"], jb["po"]
                    xa = F[:, fo: fo + 128]
                    xb = F[:, fo + 128: fo + 256]
                    w0A = F[:, fo + 256: fo + 256 + N]
                    w0B = F[:, fo + 256 + N: fo + 256 + 2 * N]
                    xp = Pp[0: bta + a, po: po + 128]
                    w1 = Pp[0: bta + a, po + 128: po + 128 + N]
                    nc.tensor.matmul(ps[:, 0:N], xa, w0A, start=True, stop=False)
                    nc.tensor.matmul(
                        ps[:, 0:N], xp[0:a, :], w1[0:a, :], start=False, stop=True
                    )
                    nc.tensor.matmul(ps[:, N: 2 * N], xb, w0B, start=True, stop=False)
                    mm = nc.tensor.matmul(
                        ps[:, N: 2 * N], xp[bta: bta + a, :], w1[bta: bta + a, :],
                        start=False, stop=True,
                    )
                elif jb["t"] == "ST":
                    N, a = jb["N"], jb["a"]
                    F = BT[jb["fu"]]
                    Pp = BT[jb["pu"]]
                    fo, po = jb["fo"], jb["po"]
                    xa = F[:, fo: fo + 128]
                    w0 = F[:, fo + 128: fo + 128 + N]
                    xp = Pp[0:a, po: po + 128]
                    w1 = Pp[0:a, po + 128: po + 128 + N]
                    nc.tensor.matmul(ps[:, 0:N], xa, w0, start=True, stop=False)
                    mm = nc.tensor.matmul(ps[:, 0:N], xp, w1, start=False, stop=True)
                else:
                    F = BT[jb["fu"]]
                    fo = jb["fo"]
                    mm = nc.tensor.matmul(
                        ps[:, 0:128], F[:, fo: fo + 128], F[:, fo + 128: fo + 256],
                        start=True, stop=True,
                    )
                mm.then_inc(P, 1)

    return nc


def _get_program():
    global _PROG
    if _PROG is None:
        _PROG = _build_program()
    return _PROG


def _pack_core(t, x_flat, W, np_bf16):
    """Build df/dp images for one core."""
    imgs = {
        "df": np.zeros((128, CF), np.float32),
        "dp": np.zeros((128, CP), np.float32),
    }
    for tn, prow, col, idx in t["xgath"]:
        blk = x_flat[:, idx].T  # [len(idx) k-rows, B cols]
        imgs[tn][prow: prow + len(idx), col: col + B] = blk
    for tn, prow, col, d, m0, m1, k0, k1 in t["wblk"]:
        imgs[tn][prow: prow + (k1 - k0), col: col + (m1 - m0)] = W[
            d, m0:m1, k0:k1
        ].T
    return {k: v.astype(np_bf16) for k, v in imgs.items()}


def kernel(x, W, b):
    import ml_dtypes
    from concourse.bass_utils import run_bass_kernel_spmd

    x = np.asarray(x, np.float32)
    W = np.asarray(W, np.float32)
    b = np.asarray(b, np.float32)
    cores, bidx = _tables()
    x_flat = x.reshape(B, S * S)
    np_bf16 = ml_dtypes.bfloat16
    in_maps = [_pack_core(t, x_flat, W, np_bf16) for t in cores]
    nc = _get_program()
    res = run_bass_kernel_spmd(nc, in_maps, core_ids=list(range(NCORES)), trace=TRACE)
    global last_results
    last_results = res
    out_flat = np.zeros((B, S * S), np.float32)
    for c, t in enumerate(cores):
        yv = np.asarray(res.results[c]["yo"], np.float32).reshape(B, -1)
        fl = t["tgt"]
        vl = fl >= 0
        out_flat[:, fl[vl]] = yv[:, vl]
    out_flat += b.reshape(-1)[bidx][None, :]
    return out_flat.reshape(B, S, S)
